# revision 18
# baseline (speedup 1.0000x reference)
"""GPT2 causal attention (B=2, T=2048, C=1024, H=16) on 8 TRN2 NeuronCores.

Sharding: core g = (batch b = g//4, head-group hg = g%4 of 4 heads).
Tensor-parallel over heads (column-split W_attn, row-split W_proj) x
data-parallel over batch. Each core computes a full [T, C] partial of the
output projection for its 4 heads; host sums the 4 partials per batch and
adds b_proj. No collectives.

Per-core kernel (bf16 matmuls, fp32 PSUM):
  qT/kT in [d, T] layout, V in [T, d] natural layout with a ones-column
  per head (so attention*V also produces the softmax row-sums). Scores are
  computed transposed, S^T[tk, tq] = kT_tile^T @ qT, exp'd without
  max-subtraction (scores ~ N(0,1)), causal tiles only, diagonal 128x128
  blocks masked with a host tri mask (left-of-diagonal junk memset to 0).
  Yu^T[d, tq] = V_aug^T @ expS^T accumulates over tk in PSUM. Row-sum
  reciprocals go through a DRAM bounce to land 128-lane for DVE recip,
  then broadcast back; yT is normalized in place and fed to the output
  projection as the stationary operand.
"""

import numpy as np
import ml_dtypes

BF16 = ml_dtypes.bfloat16

B, T, C, H, D = 2, 2048, 1024, 16, 64
HL = 4          # heads per core
DL = HL * D     # 256 local head dims
N_CORES = 8
NT = T // 128   # 16 tk tiles
NJ = T // 512   # 4 tq groups
SCALE = 1.0 / np.sqrt(D)

_CACHE = {}


def _build_program():
    import concourse.tile as tile
    from concourse import bacc
    import concourse.mybir as mybir

    f32 = mybir.dt.float32
    bf16 = mybir.dt.bfloat16
    Exp = mybir.ActivationFunctionType.Exp

    nc = bacc.Bacc("TRN2", target_bir_lowering=False, debug=False)

    # ---- DRAM I/O (host pre-sharded and pre-packed to SBUF layout) ----
    xT_d = nc.dram_tensor("xTp", [128, 8 * T], bf16, kind="ExternalInput").ap()
    wq_d = nc.dram_tensor("wqp", [128, 8 * DL], bf16, kind="ExternalInput").ap()
    wk_d = nc.dram_tensor("wkp", [128, 8 * DL], bf16, kind="ExternalInput").ap()
    wv_d = nc.dram_tensor("wvp", [128, 8 * DL], bf16, kind="ExternalInput").ap()
    wp_d = nc.dram_tensor("wpp", [128, 2 * C], bf16, kind="ExternalInput").ap()
    bq_d = nc.dram_tensor("bq", [128, 2], f32, kind="ExternalInput").ap()
    bk_d = nc.dram_tensor("bk", [128, 2], f32, kind="ExternalInput").ap()
    bvr_d = nc.dram_tensor("bvr", [128, DL], f32, kind="ExternalInput").ap()
    tri_d = nc.dram_tensor("tri", [128, 128], bf16, kind="ExternalInput").ap()
    out_d = nc.dram_tensor("out", [T, C], f32, kind="ExternalOutput").ap()
    s_dram = nc.dram_tensor("s_scratch", [HL * T], f32).ap()
    r_dram = nc.dram_tensor("r_scratch", [HL * T], bf16).ap()

    with tile.TileContext(nc) as tc:
        with (
            tc.tile_pool(name="const", bufs=1) as cpool,
            tc.tile_pool(name="exp", bufs=6) as epool,
            tc.tile_pool(name="rep", bufs=4) as rpool,
            tc.tile_pool(name="small", bufs=4) as spool,
            tc.tile_pool(name="ostage", bufs=4) as opool,
            tc.tile_pool(name="psbig", bufs=2, space="PSUM") as pbig,
            tc.tile_pool(name="psyu", bufs=4, space="PSUM") as pyu,
        ):
            # ---- persistent SBUF ----
            xT = cpool.tile([128, 8 * T], bf16, tag="xT")       # c-chunk c at [:, c*T:]
            wq = cpool.tile([128, 8 * DL], bf16, tag="wq")
            wk = cpool.tile([128, 8 * DL], bf16, tag="wk")
            wv = cpool.tile([128, 8 * DL], bf16, tag="wv")
            wp = cpool.tile([128, 2 * C], bf16, tag="wp")       # d-chunk dc at [:, dc*C:]
            bq = cpool.tile([128, 2], f32, tag="bq")
            bk = cpool.tile([128, 2], f32, tag="bk")
            bvr = cpool.tile([128, DL], f32, tag="bvr")
            tri = cpool.tile([128, 128], bf16, tag="tri")
            qT = cpool.tile([128, 2 * T], bf16, tag="qT")       # head h: [64*(h%2):, (h//2)*T + t]
            kT = cpool.tile([128, 2 * T], bf16, tag="kT")
            yT = cpool.tile([128, 2 * T], bf16, tag="yT")
            V = cpool.tile([128, NT * (HL * 65)], bf16, tag="V")  # t-tile tt, head h at [:, tt*260 + 65*h : +65]

            # ---- load inputs (few big DMAs, ordered so PE starts earliest) ----
            nc.sync.dma_start(out=wq[:, :], in_=wq_d[:, :])
            nc.sync.dma_start(out=bq[:, :], in_=bq_d[:, :])
            nc.sync.dma_start(out=xT[:, :4 * T], in_=xT_d[:, :4 * T])
            nc.sync.dma_start(out=xT[:, 4 * T:], in_=xT_d[:, 4 * T:])
            nc.sync.dma_start(out=wk[:, :], in_=wk_d[:, :])
            nc.sync.dma_start(out=bk[:, :], in_=bk_d[:, :])
            nc.sync.dma_start(out=wv[:, :], in_=wv_d[:, :])
            nc.sync.dma_start(out=bvr[:, :], in_=bvr_d[:, :])
            nc.sync.dma_start(out=tri[:, :], in_=tri_d[:, :])
            nc.sync.dma_start(out=wp[:, :], in_=wp_d[:, :])

            # ---- QKV projections ----
            for (w_sb, b_sb, dst) in ((wq, bq, qT), (wk, bk, kT)):
                for dc in range(2):
                    for ts in range(4):
                        ps = pbig.tile([128, 512], f32, tag="big")
                        for c in range(8):
                            nc.tensor.matmul(
                                ps[:, :],
                                w_sb[:, c * DL + dc * 128: c * DL + (dc + 1) * 128],
                                xT[:, c * T + ts * 512: c * T + (ts + 1) * 512],
                                start=(c == 0), stop=(c == 7),
                            )
                        nc.vector.tensor_scalar_add(
                            dst[:, dc * T + ts * 512: dc * T + (ts + 1) * 512],
                            ps[:, :], b_sb[:, dc:dc + 1],
                        )

            # V natural [t, d]: stationary xT chunk, moving W_v chunk.
            nc.vector.memset(V[:, :], 1.0)  # ones-columns; data cols overwritten
            for tt in range(NT):
                ps = pbig.tile([128, DL], f32, tag="big")
                for c in range(8):
                    nc.tensor.matmul(
                        ps[:, :],
                        xT[:, c * T + tt * 128: c * T + (tt + 1) * 128],
                        wv[:, c * DL:(c + 1) * DL],
                        start=(c == 0), stop=(c == 7),
                    )
                vdst = V[:, tt * (HL * 65): (tt + 1) * (HL * 65)].rearrange(
                    "p (h e) -> p h e", h=HL)[:, :, 0:64]
                nc.vector.tensor_add(
                    vdst,
                    ps[:, :].rearrange("p (h e) -> p h e", h=HL),
                    bvr[:, :].rearrange("p (h e) -> p h e", h=HL),
                )

            # ---- attention, head-pair interleaved, j-outer ----
            # Even/odd head score matmuls (K=64) land back-to-back with
            # tile_position rows (0,0)/(64,0), so they run concurrently in
            # the PE array's two row-group halves.
            s_view = s_dram.rearrange("(h c p) -> p h c", h=HL, c=16)
            r_view = r_dram.rearrange("(h c p) -> p h c", h=HL, c=16)
            for hp in range(2):
                fb = hp * T               # free-dim base for this head pair
                for j in range(NJ):
                    yu = [
                        pyu.tile([65, 512], f32, tag="yu", name=f"yu_{hp}_{j}_{half}")
                        for half in (0, 1)
                    ]
                    ni = 4 * j + 4        # causal: tk tiles 0..4j+3
                    for i in range(ni):
                        sc = pbig.tile([128, 1024], f32, tag="big",
                                       name=f"sc_{hp}_{j}_{i}")
                        for half in (0, 1):
                            po = 64 * half
                            nc.tensor.matmul(
                                sc[:, half * 512:(half + 1) * 512],
                                kT[po:po + 64, fb + i * 128: fb + (i + 1) * 128],
                                qT[po:po + 64, fb + j * 512: fb + (j + 1) * 512],
                                start=True, stop=True,
                            )
                        d0 = max(128 * (i - 4 * j), 0)  # diag offset in slice
                        et = epool.tile([128, 1024], bf16, tag="exp",
                                        name=f"et_{hp}_{j}_{i}")
                        et2 = et[:, :].rearrange("p (g q) -> p g q", g=2)
                        sc2 = sc[:, :].rearrange("p (g q) -> p g q", g=2)
                        nc.scalar.activation(
                            et2[:, :, d0:512], sc2[:, :, d0:512],
                            Exp, scale=float(SCALE),
                        )
                        if d0 > 0:
                            nc.vector.memset(et2[:, :, 0:d0], 0.0)
                        if i >= 4 * j:  # diagonal band: mask both halves
                            for half in (0, 1):
                                sl = slice(half * 512 + d0, half * 512 + d0 + 128)
                                nc.vector.tensor_mul(et[:, sl], et[:, sl], tri[:, :])
                        for half in (0, 1):
                            h = 2 * hp + half
                            nc.tensor.matmul(
                                yu[half][:, :],
                                V[:, i * (HL * 65) + 65 * h: i * (HL * 65) + 65 * h + 65],
                                et[:, half * 512:(half + 1) * 512],
                                start=(i == 0), stop=(i == ni - 1),
                            )
                    # Evict yu through a [65,512] fp32 stage: row 64 is the
                    # softmax denominator, rows 0-63 the unnormalized yT.
                    # DVE lanes are physical, so odd heads (po=64) cross
                    # partitions via a casting gpsimd DMA instead of DVE.
                    for half in (0, 1):
                        h = 2 * hp + half
                        stg = spool.tile([65, 512], f32, tag="stg",
                                         name=f"stg_{h}_{j}")
                        nc.vector.tensor_copy(stg[:, :], yu[half][:, :])
                        if half == 0:
                            nc.vector.tensor_copy(
                                yT[0:64, fb + j * 512: fb + (j + 1) * 512],
                                stg[0:64, :],
                            )
                        else:
                            nc.gpsimd.dma_start(
                                out=yT[64:128, fb + j * 512: fb + (j + 1) * 512],
                                in_=stg[0:64, :],
                            )
                        nc.sync.dma_start(
                            out=s_dram[h * T + j * 512: h * T + (j + 1) * 512],
                            in_=stg[64:65, :],
                        )
                    # per-(pair,j) reciprocal dance (overlaps later j's):
                    # DRAM-bounce both heads' [512] sums into [128,8] for
                    # 128-lane recip, broadcast back, normalize yT in place.
                    sT = spool.tile([128, 8], f32, tag="sT",
                                    name=f"sT_{hp}_{j}")
                    for half in (0, 1):
                        nc.sync.dma_start(
                            out=sT[:, half * 4:(half + 1) * 4],
                            in_=s_view[:, 2 * hp + half, 4 * j:4 * j + 4],
                        )
                    rT = spool.tile([128, 8], f32, tag="rT",
                                    name=f"rT_{hp}_{j}")
                    nc.vector.reciprocal(rT[:, :], sT[:, :])
                    rTb = spool.tile([128, 8], bf16, tag="rTb",
                                     name=f"rTb_{hp}_{j}")
                    nc.vector.tensor_copy(rTb[:, :], rT[:, :])
                    for half in (0, 1):
                        nc.sync.dma_start(
                            out=r_view[:, 2 * hp + half, 4 * j:4 * j + 4],
                            in_=rTb[:, half * 4:(half + 1) * 4],
                        )
                    for half in (0, 1):
                        h = 2 * hp + half
                        po = 64 * half
                        rep = rpool.tile([128, 512], bf16, tag="rep",
                                         name=f"rep_{h}_{j}")
                        nc.sync.dma_start(
                            out=rep[po:po + 64, :],
                            in_=r_dram[h * T + j * 512:
                                       h * T + (j + 1) * 512].partition_broadcast(64),
                        )
                        nc.vector.tensor_mul(
                            yT[po:po + 64, fb + j * 512: fb + (j + 1) * 512],
                            yT[po:po + 64, fb + j * 512: fb + (j + 1) * 512],
                            rep[po:po + 64, :],
                        )

            # ---- output projection: out[t, c] = sum_d yT[d, t] * wp[d, c] ----
            for tt in range(NT):
                for cc in range(2):
                    pp = pbig.tile([128, 512], f32, tag="big")
                    for dc in range(2):
                        nc.tensor.matmul(
                            pp[:, :],
                            yT[:, dc * T + tt * 128: dc * T + (tt + 1) * 128],
                            wp[:, dc * C + cc * 512: dc * C + (cc + 1) * 512],
                            start=(dc == 0), stop=(dc == 1),
                        )
                    ot = opool.tile([128, 512], f32, tag="ot")
                    if (tt + cc) % 2 == 0:
                        nc.scalar.copy(ot[:, :], pp[:, :])
                    else:
                        nc.vector.tensor_copy(ot[:, :], pp[:, :])
                    nc.sync.dma_start(
                        out=out_d[tt * 128:(tt + 1) * 128, cc * 512:(cc + 1) * 512],
                        in_=ot[:, :],
                    )

    nc.compile()
    return nc


def get_program():
    if "nc" not in _CACHE:
        _CACHE["nc"] = _build_program()
    return _CACHE["nc"]


def _pack_cmajor(a):
    """[C_rows, N] -> [128, (C_rows/128)*N] with chunk c at [:, c*N:(c+1)*N]."""
    rows, n = a.shape
    return np.ascontiguousarray(
        a.reshape(rows // 128, 128, n).transpose(1, 0, 2).reshape(128, -1))


def make_in_maps(x, W_attn, b_attn, W_proj):
    """Host-side sharding: per-core input dict."""
    x = np.asarray(x, np.float32)
    W_attn = np.asarray(W_attn, np.float32)
    b_attn = np.asarray(b_attn, np.float32)
    W_proj = np.asarray(W_proj, np.float32)

    tk = np.arange(128)[:, None]
    tq = np.arange(128)[None, :]
    tri = (tq >= tk).astype(BF16)

    xT_b = [_pack_cmajor(x[b].T.astype(BF16)) for b in range(B)]

    in_maps = []
    for g in range(N_CORES):
        b, hg = divmod(g, 4)
        cs = slice(hg * DL, (hg + 1) * DL)
        wq = _pack_cmajor(W_attn[:, 0 * C:1 * C][:, cs].astype(BF16))
        wk = _pack_cmajor(W_attn[:, 1 * C:2 * C][:, cs].astype(BF16))
        wv = _pack_cmajor(W_attn[:, 2 * C:3 * C][:, cs].astype(BF16))
        wp = _pack_cmajor(W_proj[cs, :].astype(BF16))
        bq = np.ascontiguousarray(b_attn[0 * C:1 * C][cs].reshape(2, 128).T)
        bk = np.ascontiguousarray(b_attn[1 * C:2 * C][cs].reshape(2, 128).T)
        bvr = np.ascontiguousarray(np.tile(b_attn[2 * C:3 * C][cs][None, :], (128, 1)))
        in_maps.append({
            "xTp": xT_b[b],
            "wqp": wq, "wkp": wk, "wvp": wv, "wpp": wp,
            "bq": bq.astype(np.float32), "bk": bk.astype(np.float32),
            "bvr": bvr.astype(np.float32),
            "tri": tri,
        })
    return in_maps


def assemble_output(results, b_proj):
    """results: per-core dicts with 'out' [T, C] partials."""
    b_proj = np.asarray(b_proj, np.float32)
    out = np.zeros((B, T, C), np.float32)
    for g in range(N_CORES):
        out[g // 4] += np.asarray(results[g]["out"], np.float32)
    out += b_proj[None, None, :]
    return out


def kernel(x, W_attn, b_attn, W_proj, b_proj):
    from concourse.bass_utils import run_bass_kernel_spmd

    nc = get_program()
    in_maps = make_in_maps(x, W_attn, b_attn, W_proj)
    res = run_bass_kernel_spmd(nc, in_maps, list(range(N_CORES)))
    return assemble_output(res.results, b_proj)


# revision 19
# speedup vs baseline: 13423.7872x; 13423.7872x over previous
"""GPT2 causal attention (B=2, T=2048, C=1024, H=16) on 8 TRN2 NeuronCores.

Sharding: core g = (batch b = g//4, head-group hg = g%4 of 4 heads).
Tensor-parallel over heads (column-split W_attn, row-split W_proj) x
data-parallel over batch. Each core computes a full [T, C] partial of the
output projection for its 4 heads; host sums the 4 partials per batch and
adds b_proj. No collectives.

Per-core kernel (bf16 matmuls, fp32 PSUM):
  qT/kT in [d, T] layout, V in [T, d] natural layout with a ones-column
  per head (so attention*V also produces the softmax row-sums). Scores are
  computed transposed, S^T[tk, tq] = kT_tile^T @ qT, exp'd without
  max-subtraction (scores ~ N(0,1)), causal tiles only, diagonal 128x128
  blocks masked with a host tri mask (left-of-diagonal junk memset to 0).
  Yu^T[d, tq] = V_aug^T @ expS^T accumulates over tk in PSUM. Row-sum
  reciprocals go through a DRAM bounce to land 128-lane for DVE recip,
  then broadcast back; yT is normalized in place and fed to the output
  projection as the stationary operand.
"""

import numpy as np
import ml_dtypes

BF16 = ml_dtypes.bfloat16

B, T, C, H, D = 2, 2048, 1024, 16, 64
HL = 4          # heads per core
DL = HL * D     # 256 local head dims
N_CORES = 8
NT = T // 128   # 16 tk tiles
NJ = T // 512   # 4 tq groups
SCALE = 1.0 / np.sqrt(D)

_CACHE = {}


def _build_program():
    import concourse.tile as tile
    from concourse import bacc
    import concourse.mybir as mybir

    f32 = mybir.dt.float32
    bf16 = mybir.dt.bfloat16
    Exp = mybir.ActivationFunctionType.Exp

    nc = bacc.Bacc("TRN2", target_bir_lowering=False, debug=False)

    # ---- DRAM I/O (host pre-sharded and pre-packed to SBUF layout) ----
    xT_d = nc.dram_tensor("xTp", [128, 8 * T], bf16, kind="ExternalInput").ap()
    wq_d = nc.dram_tensor("wqp", [128, 8 * DL], bf16, kind="ExternalInput").ap()
    wk_d = nc.dram_tensor("wkp", [128, 8 * DL], bf16, kind="ExternalInput").ap()
    wv_d = nc.dram_tensor("wvp", [128, 8 * DL], bf16, kind="ExternalInput").ap()
    wp_d = nc.dram_tensor("wpp", [128, 2 * C], bf16, kind="ExternalInput").ap()
    bq_d = nc.dram_tensor("bq", [128, 2], f32, kind="ExternalInput").ap()
    bk_d = nc.dram_tensor("bk", [128, 2], f32, kind="ExternalInput").ap()
    bvr_d = nc.dram_tensor("bvr", [128, DL], f32, kind="ExternalInput").ap()
    tri_d = nc.dram_tensor("tri", [128, 128], bf16, kind="ExternalInput").ap()
    out_d = nc.dram_tensor("out", [T, C], f32, kind="ExternalOutput").ap()
    s_dram = nc.dram_tensor("s_scratch", [HL * T], f32).ap()
    r_dram = nc.dram_tensor("r_scratch", [HL * T], bf16).ap()

    with tile.TileContext(nc) as tc:
        with (
            tc.tile_pool(name="const", bufs=1) as cpool,
            tc.tile_pool(name="exp", bufs=6) as epool,
            tc.tile_pool(name="rep", bufs=4) as rpool,
            tc.tile_pool(name="small", bufs=4) as spool,
            tc.tile_pool(name="ostage", bufs=4) as opool,
            tc.tile_pool(name="psbig", bufs=2, space="PSUM") as pbig,
            tc.tile_pool(name="psyu", bufs=4, space="PSUM") as pyu,
        ):
            # ---- persistent SBUF ----
            xT = cpool.tile([128, 8 * T], bf16, tag="xT")       # c-chunk c at [:, c*T:]
            wq = cpool.tile([128, 8 * DL], bf16, tag="wq")
            wk = cpool.tile([128, 8 * DL], bf16, tag="wk")
            wv = cpool.tile([128, 8 * DL], bf16, tag="wv")
            wp = cpool.tile([128, 2 * C], bf16, tag="wp")       # d-chunk dc at [:, dc*C:]
            bq = cpool.tile([128, 2], f32, tag="bq")
            bk = cpool.tile([128, 2], f32, tag="bk")
            bvr = cpool.tile([128, DL], f32, tag="bvr")
            tri = cpool.tile([128, 128], bf16, tag="tri")
            qT = cpool.tile([128, 2 * T], bf16, tag="qT")       # head h: [64*(h%2):, (h//2)*T + t]
            kT = cpool.tile([128, 2 * T], bf16, tag="kT")
            yT = cpool.tile([128, 2 * T], bf16, tag="yT")
            V = cpool.tile([128, NT * (HL * 65)], bf16, tag="V")  # t-tile tt, head h at [:, tt*260 + 65*h : +65]

            # ---- load inputs (few big DMAs, ordered so PE starts earliest) ----
            nc.sync.dma_start(out=wq[:, :], in_=wq_d[:, :])
            nc.sync.dma_start(out=bq[:, :], in_=bq_d[:, :])
            for c in range(8):  # per-chunk so the first QKV matmuls start early
                nc.sync.dma_start(out=xT[:, c * T:(c + 1) * T],
                                  in_=xT_d[:, c * T:(c + 1) * T])
            nc.sync.dma_start(out=wk[:, :], in_=wk_d[:, :])
            nc.sync.dma_start(out=bk[:, :], in_=bk_d[:, :])
            nc.sync.dma_start(out=wv[:, :], in_=wv_d[:, :])
            nc.sync.dma_start(out=bvr[:, :], in_=bvr_d[:, :])
            nc.sync.dma_start(out=tri[:, :], in_=tri_d[:, :])
            nc.sync.dma_start(out=wp[:, :], in_=wp_d[:, :])

            # ---- QKV projections ----
            for (w_sb, b_sb, dst) in ((wq, bq, qT), (wk, bk, kT)):
                for dc in range(2):
                    for ts in range(4):
                        ps = pbig.tile([128, 512], f32, tag="big")
                        for c in range(8):
                            nc.tensor.matmul(
                                ps[:, :],
                                w_sb[:, c * DL + dc * 128: c * DL + (dc + 1) * 128],
                                xT[:, c * T + ts * 512: c * T + (ts + 1) * 512],
                                start=(c == 0), stop=(c == 7),
                            )
                        nc.vector.tensor_scalar_add(
                            dst[:, dc * T + ts * 512: dc * T + (ts + 1) * 512],
                            ps[:, :], b_sb[:, dc:dc + 1],
                        )

            # V natural [t, d]: stationary xT chunk, moving W_v chunk.
            nc.vector.memset(V[:, :], 1.0)  # ones-columns; data cols overwritten
            for tt in range(NT):
                ps = pbig.tile([128, DL], f32, tag="big")
                for c in range(8):
                    nc.tensor.matmul(
                        ps[:, :],
                        xT[:, c * T + tt * 128: c * T + (tt + 1) * 128],
                        wv[:, c * DL:(c + 1) * DL],
                        start=(c == 0), stop=(c == 7),
                    )
                vdst = V[:, tt * (HL * 65): (tt + 1) * (HL * 65)].rearrange(
                    "p (h e) -> p h e", h=HL)[:, :, 0:64]
                nc.vector.tensor_add(
                    vdst,
                    ps[:, :].rearrange("p (h e) -> p h e", h=HL),
                    bvr[:, :].rearrange("p (h e) -> p h e", h=HL),
                )

            # ---- attention, head-pair interleaved, j-outer ----
            # Even/odd head score matmuls (K=64) land back-to-back with
            # tile_position rows (0,0)/(64,0), so they run concurrently in
            # the PE array's two row-group halves.
            s_view = s_dram.rearrange("(h c p) -> p h c", h=HL, c=16)
            r_view = r_dram.rearrange("(h c p) -> p h c", h=HL, c=16)
            for hp in range(2):
                fb = hp * T               # free-dim base for this head pair
                for j in range(NJ):
                    yu = [
                        pyu.tile([65, 512], f32, tag="yu", name=f"yu_{hp}_{j}_{half}")
                        for half in (0, 1)
                    ]
                    ni = 4 * j + 4        # causal: tk tiles 0..4j+3
                    for i in range(ni):
                        sc = pbig.tile([128, 1024], f32, tag="big",
                                       name=f"sc_{hp}_{j}_{i}")
                        for half in (0, 1):
                            po = 64 * half
                            nc.tensor.matmul(
                                sc[:, half * 512:(half + 1) * 512],
                                kT[po:po + 64, fb + i * 128: fb + (i + 1) * 128],
                                qT[po:po + 64, fb + j * 512: fb + (j + 1) * 512],
                                start=True, stop=True,
                            )
                        d0 = max(128 * (i - 4 * j), 0)  # diag offset in slice
                        et = epool.tile([128, 1024], bf16, tag="exp",
                                        name=f"et_{hp}_{j}_{i}")
                        et2 = et[:, :].rearrange("p (g q) -> p g q", g=2)
                        sc2 = sc[:, :].rearrange("p (g q) -> p g q", g=2)
                        nc.scalar.activation(
                            et2[:, :, d0:512], sc2[:, :, d0:512],
                            Exp, scale=float(SCALE),
                        )
                        if d0 > 0:
                            nc.vector.memset(et2[:, :, 0:d0], 0.0)
                        if i >= 4 * j:  # diagonal band: mask both halves
                            for half in (0, 1):
                                sl = slice(half * 512 + d0, half * 512 + d0 + 128)
                                nc.vector.tensor_mul(et[:, sl], et[:, sl], tri[:, :])
                        for half in (0, 1):
                            h = 2 * hp + half
                            nc.tensor.matmul(
                                yu[half][:, :],
                                V[:, i * (HL * 65) + 65 * h: i * (HL * 65) + 65 * h + 65],
                                et[:, half * 512:(half + 1) * 512],
                                start=(i == 0), stop=(i == ni - 1),
                            )
                    # Evict yu through a [65,512] fp32 stage: row 64 is the
                    # softmax denominator, rows 0-63 the unnormalized yT.
                    # DVE lanes are physical, so odd heads (po=64) cross
                    # partitions via a casting gpsimd DMA instead of DVE.
                    for half in (0, 1):
                        h = 2 * hp + half
                        stg = spool.tile([65, 512], f32, tag="stg",
                                         name=f"stg_{h}_{j}")
                        nc.vector.tensor_copy(stg[:, :], yu[half][:, :])
                        if half == 0:
                            nc.vector.tensor_copy(
                                yT[0:64, fb + j * 512: fb + (j + 1) * 512],
                                stg[0:64, :],
                            )
                        else:
                            nc.gpsimd.dma_start(
                                out=yT[64:128, fb + j * 512: fb + (j + 1) * 512],
                                in_=stg[0:64, :],
                            )
                        nc.sync.dma_start(
                            out=s_dram[h * T + j * 512: h * T + (j + 1) * 512],
                            in_=stg[64:65, :],
                        )
                    # per-(pair,j) reciprocal dance (overlaps later j's):
                    # DRAM-bounce both heads' [512] sums into [128,8] for
                    # 128-lane recip, broadcast back, normalize yT in place.
                    sT = spool.tile([128, 8], f32, tag="sT",
                                    name=f"sT_{hp}_{j}")
                    for half in (0, 1):
                        nc.sync.dma_start(
                            out=sT[:, half * 4:(half + 1) * 4],
                            in_=s_view[:, 2 * hp + half, 4 * j:4 * j + 4],
                        )
                    rT = spool.tile([128, 8], f32, tag="rT",
                                    name=f"rT_{hp}_{j}")
                    nc.vector.reciprocal(rT[:, :], sT[:, :])
                    rTb = spool.tile([128, 8], bf16, tag="rTb",
                                     name=f"rTb_{hp}_{j}")
                    nc.vector.tensor_copy(rTb[:, :], rT[:, :])
                    for half in (0, 1):
                        nc.sync.dma_start(
                            out=r_view[:, 2 * hp + half, 4 * j:4 * j + 4],
                            in_=rTb[:, half * 4:(half + 1) * 4],
                        )
                    for half in (0, 1):
                        h = 2 * hp + half
                        po = 64 * half
                        rep = rpool.tile([128, 512], bf16, tag="rep",
                                         name=f"rep_{h}_{j}")
                        nc.sync.dma_start(
                            out=rep[po:po + 64, :],
                            in_=r_dram[h * T + j * 512:
                                       h * T + (j + 1) * 512].partition_broadcast(64),
                        )
                        nc.vector.tensor_mul(
                            yT[po:po + 64, fb + j * 512: fb + (j + 1) * 512],
                            yT[po:po + 64, fb + j * 512: fb + (j + 1) * 512],
                            rep[po:po + 64, :],
                        )

            # ---- output projection: out[t, c] = sum_d yT[d, t] * wp[d, c] ----
            for tt in range(NT):
                for cc in range(2):
                    pp = pbig.tile([128, 512], f32, tag="big")
                    for dc in range(2):
                        nc.tensor.matmul(
                            pp[:, :],
                            yT[:, dc * T + tt * 128: dc * T + (tt + 1) * 128],
                            wp[:, dc * C + cc * 512: dc * C + (cc + 1) * 512],
                            start=(dc == 0), stop=(dc == 1),
                        )
                    ot = opool.tile([128, 512], f32, tag="ot")
                    if (tt + cc) % 2 == 0:
                        nc.scalar.copy(ot[:, :], pp[:, :])
                    else:
                        nc.vector.tensor_copy(ot[:, :], pp[:, :])
                    nc.sync.dma_start(
                        out=out_d[tt * 128:(tt + 1) * 128, cc * 512:(cc + 1) * 512],
                        in_=ot[:, :],
                    )

    nc.compile()
    return nc


def get_program():
    if "nc" not in _CACHE:
        _CACHE["nc"] = _build_program()
    return _CACHE["nc"]


def _pack_cmajor(a):
    """[C_rows, N] -> [128, (C_rows/128)*N] with chunk c at [:, c*N:(c+1)*N]."""
    rows, n = a.shape
    return np.ascontiguousarray(
        a.reshape(rows // 128, 128, n).transpose(1, 0, 2).reshape(128, -1))


def make_in_maps(x, W_attn, b_attn, W_proj):
    """Host-side sharding: per-core input dict."""
    x = np.asarray(x, np.float32)
    W_attn = np.asarray(W_attn, np.float32)
    b_attn = np.asarray(b_attn, np.float32)
    W_proj = np.asarray(W_proj, np.float32)

    tk = np.arange(128)[:, None]
    tq = np.arange(128)[None, :]
    tri = (tq >= tk).astype(BF16)

    xT_b = [_pack_cmajor(x[b].T.astype(BF16)) for b in range(B)]

    in_maps = []
    for g in range(N_CORES):
        b, hg = divmod(g, 4)
        cs = slice(hg * DL, (hg + 1) * DL)
        wq = _pack_cmajor(W_attn[:, 0 * C:1 * C][:, cs].astype(BF16))
        wk = _pack_cmajor(W_attn[:, 1 * C:2 * C][:, cs].astype(BF16))
        wv = _pack_cmajor(W_attn[:, 2 * C:3 * C][:, cs].astype(BF16))
        wp = _pack_cmajor(W_proj[cs, :].astype(BF16))
        bq = np.ascontiguousarray(b_attn[0 * C:1 * C][cs].reshape(2, 128).T)
        bk = np.ascontiguousarray(b_attn[1 * C:2 * C][cs].reshape(2, 128).T)
        bvr = np.ascontiguousarray(np.tile(b_attn[2 * C:3 * C][cs][None, :], (128, 1)))
        in_maps.append({
            "xTp": xT_b[b],
            "wqp": wq, "wkp": wk, "wvp": wv, "wpp": wp,
            "bq": bq.astype(np.float32), "bk": bk.astype(np.float32),
            "bvr": bvr.astype(np.float32),
            "tri": tri,
        })
    return in_maps


def assemble_output(results, b_proj):
    """results: per-core dicts with 'out' [T, C] partials."""
    b_proj = np.asarray(b_proj, np.float32)
    out = np.zeros((B, T, C), np.float32)
    for g in range(N_CORES):
        out[g // 4] += np.asarray(results[g]["out"], np.float32)
    out += b_proj[None, None, :]
    return out


def kernel(x, W_attn, b_attn, W_proj, b_proj):
    from concourse.bass_utils import run_bass_kernel_spmd

    nc = get_program()
    in_maps = make_in_maps(x, W_attn, b_attn, W_proj)
    res = run_bass_kernel_spmd(nc, in_maps, list(range(N_CORES)))
    return assemble_output(res.results, b_proj)


# revision 20
# speedup vs baseline: 13844.5115x; 1.0313x over previous
"""GPT2 causal attention (B=2, T=2048, C=1024, H=16) on 8 TRN2 NeuronCores.

Sharding: core g = (batch b = g//4, head-group hg = g%4 of 4 heads).
Tensor-parallel over heads (column-split W_attn, row-split W_proj) x
data-parallel over batch. Each core computes a full [T, C] partial of the
output projection for its 4 heads; host sums the 4 partials per batch and
adds b_proj. No collectives.

Per-core kernel (bf16 matmuls, fp32 PSUM):
  qT/kT in [d, T] layout, V in [T, d] natural layout with a ones-column
  per head (so attention*V also produces the softmax row-sums). Scores are
  computed transposed, S^T[tk, tq] = kT_tile^T @ qT, exp'd without
  max-subtraction (scores ~ N(0,1)), causal tiles only, diagonal 128x128
  blocks masked with a host tri mask (left-of-diagonal junk memset to 0).
  Yu^T[d, tq] = V_aug^T @ expS^T accumulates over tk in PSUM. Row-sum
  reciprocals go through a DRAM bounce to land 128-lane for DVE recip,
  then broadcast back; yT is normalized in place and fed to the output
  projection as the stationary operand.
"""

import numpy as np
import ml_dtypes

BF16 = ml_dtypes.bfloat16

B, T, C, H, D = 2, 2048, 1024, 16, 64
HL = 4          # heads per core
DL = HL * D     # 256 local head dims
N_CORES = 8
NT = T // 128   # 16 tk tiles
NJ = T // 512   # 4 tq groups
SCALE = 1.0 / np.sqrt(D)

_CACHE = {}


def _build_program():
    import concourse.tile as tile
    from concourse import bacc
    import concourse.mybir as mybir

    f32 = mybir.dt.float32
    bf16 = mybir.dt.bfloat16
    Exp = mybir.ActivationFunctionType.Exp

    nc = bacc.Bacc("TRN2", target_bir_lowering=False, debug=False)

    # ---- DRAM I/O (host pre-sharded and pre-packed to SBUF layout) ----
    xT_d = nc.dram_tensor("xTp", [128, 8 * T], bf16, kind="ExternalInput").ap()
    wq_d = nc.dram_tensor("wqp", [128, 8 * DL], bf16, kind="ExternalInput").ap()
    wk_d = nc.dram_tensor("wkp", [128, 8 * DL], bf16, kind="ExternalInput").ap()
    wv_d = nc.dram_tensor("wvp", [128, 8 * DL], bf16, kind="ExternalInput").ap()
    wp_d = nc.dram_tensor("wpp", [128, 2 * C], bf16, kind="ExternalInput").ap()
    bq_d = nc.dram_tensor("bq", [128, 2], f32, kind="ExternalInput").ap()
    bk_d = nc.dram_tensor("bk", [128, 2], f32, kind="ExternalInput").ap()
    bvr_d = nc.dram_tensor("bvr", [128, DL], f32, kind="ExternalInput").ap()
    tri_d = nc.dram_tensor("tri", [128, 128], bf16, kind="ExternalInput").ap()
    out_d = nc.dram_tensor("out", [T, C], f32, kind="ExternalOutput").ap()
    s_dram = nc.dram_tensor("s_scratch", [HL * T], f32).ap()
    r_dram = nc.dram_tensor("r_scratch", [HL * T], bf16).ap()

    with tile.TileContext(nc) as tc:
        with (
            tc.tile_pool(name="const", bufs=1) as cpool,
            tc.tile_pool(name="exp", bufs=8) as epool,
            tc.tile_pool(name="rep", bufs=6) as rpool,
            tc.tile_pool(name="small", bufs=6) as spool,
            tc.tile_pool(name="ostage", bufs=6) as opool,
            tc.tile_pool(name="psbig", bufs=2, space="PSUM") as pbig,
            tc.tile_pool(name="psyu", bufs=4, space="PSUM") as pyu,
        ):
            # ---- persistent SBUF ----
            xT = cpool.tile([128, 8 * T], bf16, tag="xT")       # c-chunk c at [:, c*T:]
            wq = cpool.tile([128, 8 * DL], bf16, tag="wq")
            wk = cpool.tile([128, 8 * DL], bf16, tag="wk")
            wv = cpool.tile([128, 8 * DL], bf16, tag="wv")
            wp = cpool.tile([128, 2 * C], bf16, tag="wp")       # d-chunk dc at [:, dc*C:]
            bq = cpool.tile([128, 2], f32, tag="bq")
            bk = cpool.tile([128, 2], f32, tag="bk")
            bvr = cpool.tile([128, DL], f32, tag="bvr")
            tri = cpool.tile([128, 128], bf16, tag="tri")
            qT = cpool.tile([128, 2 * T], bf16, tag="qT")       # head h: [64*(h%2):, (h//2)*T + t]
            kT = cpool.tile([128, 2 * T], bf16, tag="kT")
            yT = cpool.tile([128, 2 * T], bf16, tag="yT")
            V = cpool.tile([128, NT * (HL * 65)], bf16, tag="V")  # t-tile tt, head h at [:, tt*260 + 65*h : +65]

            # ---- load inputs (few big DMAs, ordered so PE starts earliest) ----
            nc.sync.dma_start(out=wq[:, :], in_=wq_d[:, :])
            nc.sync.dma_start(out=bq[:, :], in_=bq_d[:, :])
            for c in range(8):  # per-chunk so the first QKV matmuls start early
                nc.sync.dma_start(out=xT[:, c * T:(c + 1) * T],
                                  in_=xT_d[:, c * T:(c + 1) * T])
            nc.sync.dma_start(out=wk[:, :], in_=wk_d[:, :])
            nc.sync.dma_start(out=bk[:, :], in_=bk_d[:, :])
            nc.sync.dma_start(out=wv[:, :], in_=wv_d[:, :])
            nc.sync.dma_start(out=bvr[:, :], in_=bvr_d[:, :])
            nc.sync.dma_start(out=tri[:, :], in_=tri_d[:, :])
            nc.sync.dma_start(out=wp[:, :], in_=wp_d[:, :])

            # ---- QKV projections ----
            for (w_sb, b_sb, dst) in ((wq, bq, qT), (wk, bk, kT)):
                for dc in range(2):
                    for ts in range(4):
                        ps = pbig.tile([128, 512], f32, tag="big")
                        for c in range(8):
                            nc.tensor.matmul(
                                ps[:, :],
                                w_sb[:, c * DL + dc * 128: c * DL + (dc + 1) * 128],
                                xT[:, c * T + ts * 512: c * T + (ts + 1) * 512],
                                start=(c == 0), stop=(c == 7),
                            )
                        nc.vector.tensor_scalar_add(
                            dst[:, dc * T + ts * 512: dc * T + (ts + 1) * 512],
                            ps[:, :], b_sb[:, dc:dc + 1],
                        )

            # V natural [t, d]: stationary xT chunk, moving W_v chunk.
            nc.vector.memset(V[:, :], 1.0)  # ones-columns; data cols overwritten
            for tt in range(NT):
                ps = pbig.tile([128, DL], f32, tag="big")
                for c in range(8):
                    nc.tensor.matmul(
                        ps[:, :],
                        xT[:, c * T + tt * 128: c * T + (tt + 1) * 128],
                        wv[:, c * DL:(c + 1) * DL],
                        start=(c == 0), stop=(c == 7),
                    )
                vdst = V[:, tt * (HL * 65): (tt + 1) * (HL * 65)].rearrange(
                    "p (h e) -> p h e", h=HL)[:, :, 0:64]
                nc.vector.tensor_add(
                    vdst,
                    ps[:, :].rearrange("p (h e) -> p h e", h=HL),
                    bvr[:, :].rearrange("p (h e) -> p h e", h=HL),
                )

            # ---- attention, head-pair interleaved, j-outer ----
            # Even/odd head score matmuls (K=64) land back-to-back with
            # tile_position rows (0,0)/(64,0), so they run concurrently in
            # the PE array's two row-group halves.
            s_view = s_dram.rearrange("(h c p) -> p h c", h=HL, c=16)
            r_view = r_dram.rearrange("(h c p) -> p h c", h=HL, c=16)
            for hp in range(2):
                fb = hp * T               # free-dim base for this head pair
                for j in range(NJ):
                    yu = [
                        pyu.tile([65, 512], f32, tag="yu", name=f"yu_{hp}_{j}_{half}")
                        for half in (0, 1)
                    ]
                    ni = 4 * j + 4        # causal: tk tiles 0..4j+3
                    for i in range(ni):
                        sc = pbig.tile([128, 1024], f32, tag="big",
                                       name=f"sc_{hp}_{j}_{i}")
                        for half in (0, 1):
                            po = 64 * half
                            nc.tensor.matmul(
                                sc[:, half * 512:(half + 1) * 512],
                                kT[po:po + 64, fb + i * 128: fb + (i + 1) * 128],
                                qT[po:po + 64, fb + j * 512: fb + (j + 1) * 512],
                                start=True, stop=True,
                            )
                        d0 = max(128 * (i - 4 * j), 0)  # diag offset in slice
                        et = epool.tile([128, 1024], bf16, tag="exp",
                                        name=f"et_{hp}_{j}_{i}")
                        et2 = et[:, :].rearrange("p (g q) -> p g q", g=2)
                        sc2 = sc[:, :].rearrange("p (g q) -> p g q", g=2)
                        nc.scalar.activation(
                            et2[:, :, d0:512], sc2[:, :, d0:512],
                            Exp, scale=float(SCALE),
                        )
                        if d0 > 0:
                            nc.vector.memset(et2[:, :, 0:d0], 0.0)
                        if i >= 4 * j:  # diagonal band: mask both halves
                            for half in (0, 1):
                                sl = slice(half * 512 + d0, half * 512 + d0 + 128)
                                nc.vector.tensor_mul(et[:, sl], et[:, sl], tri[:, :])
                        for half in (0, 1):
                            h = 2 * hp + half
                            nc.tensor.matmul(
                                yu[half][:, :],
                                V[:, i * (HL * 65) + 65 * h: i * (HL * 65) + 65 * h + 65],
                                et[:, half * 512:(half + 1) * 512],
                                start=(i == 0), stop=(i == ni - 1),
                            )
                    # Evict yu through a [65,512] fp32 stage: row 64 is the
                    # softmax denominator, rows 0-63 the unnormalized yT.
                    # DVE lanes are physical, so odd heads (po=64) cross
                    # partitions via a casting gpsimd DMA instead of DVE.
                    for half in (0, 1):
                        h = 2 * hp + half
                        stg = spool.tile([65, 512], f32, tag="stg",
                                         name=f"stg_{h}_{j}")
                        nc.vector.tensor_copy(stg[:, :], yu[half][:, :])
                        if half == 0:
                            nc.vector.tensor_copy(
                                yT[0:64, fb + j * 512: fb + (j + 1) * 512],
                                stg[0:64, :],
                            )
                        else:
                            nc.gpsimd.dma_start(
                                out=yT[64:128, fb + j * 512: fb + (j + 1) * 512],
                                in_=stg[0:64, :],
                            )
                        nc.sync.dma_start(
                            out=s_dram[h * T + j * 512: h * T + (j + 1) * 512],
                            in_=stg[64:65, :],
                        )
                    # per-(pair,j) reciprocal dance (overlaps later j's):
                    # DRAM-bounce both heads' [512] sums into [128,8] for
                    # 128-lane recip, broadcast back, normalize yT in place.
                    sT = spool.tile([128, 8], f32, tag="sT",
                                    name=f"sT_{hp}_{j}")
                    for half in (0, 1):
                        nc.sync.dma_start(
                            out=sT[:, half * 4:(half + 1) * 4],
                            in_=s_view[:, 2 * hp + half, 4 * j:4 * j + 4],
                        )
                    rT = spool.tile([128, 8], f32, tag="rT",
                                    name=f"rT_{hp}_{j}")
                    nc.vector.reciprocal(rT[:, :], sT[:, :])
                    rTb = spool.tile([128, 8], bf16, tag="rTb",
                                     name=f"rTb_{hp}_{j}")
                    nc.vector.tensor_copy(rTb[:, :], rT[:, :])
                    for half in (0, 1):
                        nc.sync.dma_start(
                            out=r_view[:, 2 * hp + half, 4 * j:4 * j + 4],
                            in_=rTb[:, half * 4:(half + 1) * 4],
                        )
                    for half in (0, 1):
                        h = 2 * hp + half
                        po = 64 * half
                        rep = rpool.tile([128, 512], bf16, tag="rep",
                                         name=f"rep_{h}_{j}")
                        nc.sync.dma_start(
                            out=rep[po:po + 64, :],
                            in_=r_dram[h * T + j * 512:
                                       h * T + (j + 1) * 512].partition_broadcast(64),
                        )
                        nc.vector.tensor_mul(
                            yT[po:po + 64, fb + j * 512: fb + (j + 1) * 512],
                            yT[po:po + 64, fb + j * 512: fb + (j + 1) * 512],
                            rep[po:po + 64, :],
                        )

            # ---- output projection: out[t, c] = sum_d yT[d, t] * wp[d, c] ----
            for tt in range(NT):
                for cc in range(2):
                    pp = pbig.tile([128, 512], f32, tag="big")
                    for dc in range(2):
                        nc.tensor.matmul(
                            pp[:, :],
                            yT[:, dc * T + tt * 128: dc * T + (tt + 1) * 128],
                            wp[:, dc * C + cc * 512: dc * C + (cc + 1) * 512],
                            start=(dc == 0), stop=(dc == 1),
                        )
                    ot = opool.tile([128, 512], f32, tag="ot")
                    if (tt + cc) % 2 == 0:
                        nc.scalar.copy(ot[:, :], pp[:, :])
                    else:
                        nc.vector.tensor_copy(ot[:, :], pp[:, :])
                    nc.sync.dma_start(
                        out=out_d[tt * 128:(tt + 1) * 128, cc * 512:(cc + 1) * 512],
                        in_=ot[:, :],
                    )

    nc.compile()
    return nc


def get_program():
    if "nc" not in _CACHE:
        _CACHE["nc"] = _build_program()
    return _CACHE["nc"]


def _pack_cmajor(a):
    """[C_rows, N] -> [128, (C_rows/128)*N] with chunk c at [:, c*N:(c+1)*N]."""
    rows, n = a.shape
    return np.ascontiguousarray(
        a.reshape(rows // 128, 128, n).transpose(1, 0, 2).reshape(128, -1))


def make_in_maps(x, W_attn, b_attn, W_proj):
    """Host-side sharding: per-core input dict."""
    x = np.asarray(x, np.float32)
    W_attn = np.asarray(W_attn, np.float32)
    b_attn = np.asarray(b_attn, np.float32)
    W_proj = np.asarray(W_proj, np.float32)

    tk = np.arange(128)[:, None]
    tq = np.arange(128)[None, :]
    tri = (tq >= tk).astype(BF16)

    xT_b = [_pack_cmajor(x[b].T.astype(BF16)) for b in range(B)]

    in_maps = []
    for g in range(N_CORES):
        b, hg = divmod(g, 4)
        cs = slice(hg * DL, (hg + 1) * DL)
        wq = _pack_cmajor(W_attn[:, 0 * C:1 * C][:, cs].astype(BF16))
        wk = _pack_cmajor(W_attn[:, 1 * C:2 * C][:, cs].astype(BF16))
        wv = _pack_cmajor(W_attn[:, 2 * C:3 * C][:, cs].astype(BF16))
        wp = _pack_cmajor(W_proj[cs, :].astype(BF16))
        bq = np.ascontiguousarray(b_attn[0 * C:1 * C][cs].reshape(2, 128).T)
        bk = np.ascontiguousarray(b_attn[1 * C:2 * C][cs].reshape(2, 128).T)
        bvr = np.ascontiguousarray(np.tile(b_attn[2 * C:3 * C][cs][None, :], (128, 1)))
        in_maps.append({
            "xTp": xT_b[b],
            "wqp": wq, "wkp": wk, "wvp": wv, "wpp": wp,
            "bq": bq.astype(np.float32), "bk": bk.astype(np.float32),
            "bvr": bvr.astype(np.float32),
            "tri": tri,
        })
    return in_maps


def assemble_output(results, b_proj):
    """results: per-core dicts with 'out' [T, C] partials."""
    b_proj = np.asarray(b_proj, np.float32)
    out = np.zeros((B, T, C), np.float32)
    for g in range(N_CORES):
        out[g // 4] += np.asarray(results[g]["out"], np.float32)
    out += b_proj[None, None, :]
    return out


def kernel(x, W_attn, b_attn, W_proj, b_proj):
    from concourse.bass_utils import run_bass_kernel_spmd

    nc = get_program()
    in_maps = make_in_maps(x, W_attn, b_attn, W_proj)
    res = run_bass_kernel_spmd(nc, in_maps, list(range(N_CORES)))
    return assemble_output(res.results, b_proj)
